# revision 15
# baseline (speedup 1.0000x reference)
"""Distributed Bass kernel for nn_Attention_12953621365048 (8 TRN2 NeuronCores).

Sharding: 2 batch-groups x 4 head-groups (3 heads/core); core c handles
batch b=c//4, heads 3*(c%4)..3*(c%4)+2.

Fused per-head pipeline: the QKV projection + norms of head t+1 are
emitted as "filler" pieces inside the ACT-bound attention stream of
head t so the PE never idles; the V projection for all heads is zipped
per-kv-tile into attention(head 0)'s first q-chunk.

RMSNorm: per-chunk sum-of-squares rows are packed into one [1, 4608]
SBUF row (partition 0) at free offsets, so a single Abs_reciprocal_sqrt
per head computes every inv (2 activation-table loads per head, between
the exp streams); normalization is applied in place on kn/qn before an
in-place rope whose swap copies ride the DVE (partition-shifted copies
are legal, shifted arithmetic is not).

A2A: each core writes its attention output twice (blocks qc and 4+qc),
masked by per-core 0/1 scalars (batch 0 cores zero the upper blocks,
batch 1 cores the lower) folded into the output-normalization multiply.
Every destination sums block pairs j/(4+j) -- one real, one zero --
giving a uniform program with a 4-block (unpadded) projection.
"""

import math
from contextlib import ExitStack

import numpy as np
import ml_dtypes

import concourse.mybir as mybir
import concourse.tile as tile
from concourse import bacc
from concourse.bass_utils import run_bass_kernel_spmd

B, N, M, C, H, HD, RD = 2, 2048, 512, 1536, 12, 128, 64
EPS = 1e-6
NHL = 3               # heads per core
S = N + M             # 2560 kv tokens
KT = S // 128         # 20 kv tiles
NXT = N // 128        # 16 x kv tiles
NQC = N // 512        # 4 q-chunks of 512
CH = 512              # qkv-phase token chunk
NKC = 5               # k chunks (4 x + 1 y)
NCT = C // 128        # 12 contraction tiles
ROWW = S + N          # 4608: k rows at [0,S), q rows at [S, S+N)
F32 = mybir.dt.float32
BF16 = mybir.dt.bfloat16
AF = mybir.ActivationFunctionType
ALU = mybir.AluOpType


def build_nc():
    nc = bacc.Bacc("TRN2", target_bir_lowering=False, debug=False, num_devices=8)

    xT = nc.dram_tensor("xT", [C, N], BF16, kind="ExternalInput").ap()
    yT = nc.dram_tensor("yT", [C, M], BF16, kind="ExternalInput").ap()
    wqk = nc.dram_tensor("wqk", [C, NHL * 256], BF16, kind="ExternalInput").ap()
    wky = nc.dram_tensor("wky", [C, NHL * 128], BF16, kind="ExternalInput").ap()
    wvx = nc.dram_tensor("wvx", [C, NHL * HD], BF16, kind="ExternalInput").ap()
    wvy = nc.dram_tensor("wvy", [C, NHL * HD], BF16, kind="ExternalInput").ap()
    # proj rows grouped [t][i] = Wproj rows of global head 3i+t (batch indep.)
    wproj = nc.dram_tensor("wproj", [NHL * 4 * 128, C], BF16, kind="ExternalInput").ap()
    cs = nc.dram_tensor("cs", [RD, N], BF16, kind="ExternalInput").ap()
    sn = nc.dram_tensor("sn", [RD, N], BF16, kind="ExternalInput").ap()
    ywT = nc.dram_tensor("ywT", [128, M // 128], F32, kind="ExternalInput").ap()
    bpr = nc.dram_tensor("bpr", [1, C], BF16, kind="ExternalInput").ap()
    onesb = nc.dram_tensor("onesb", [128, 1], BF16, kind="ExternalInput").ap()
    mska = nc.dram_tensor("mska", [128, 1], F32, kind="ExternalInput").ap()
    mskb = nc.dram_tensor("mskb", [128, 1], F32, kind="ExternalInput").ap()
    out = nc.dram_tensor("out", [512, C], F32, kind="ExternalOutput").ap()

    with tile.TileContext(nc) as tc, ExitStack() as ctx:
        pers = ctx.enter_context(tc.tile_pool(name="persist", bufs=1))
        dram = ctx.enter_context(tc.tile_pool(name="dram", bufs=1, space="DRAM"))

        onesb_sb = pers.tile([128, 1], BF16, tag="onesb")
        nc.sync.dma_start(onesb_sb[:], onesb)
        mska_sb = pers.tile([128, 1], F32, tag="mska")
        nc.sync.dma_start(mska_sb[:], mska)
        mskb_sb = pers.tile([128, 1], F32, tag="mskb")
        nc.sync.dma_start(mskb_sb[:], mskb)
        hdm_sb = pers.tile([128, 1], F32, tag="hdm")
        nc.vector.memset(hdm_sb[:], HD ** -0.5)
        z128 = pers.tile([128, 1], F32, tag="z128")
        nc.vector.memset(z128[:], 0.0)
        eps1 = pers.tile([1, 1], F32, tag="eps1")
        nc.vector.memset(eps1[:], EPS)

        # attention bias per kv tile column: 0 for x tiles, ln(clip(w)) for y
        bias_sb = pers.tile([128, KT], F32, tag="bias")
        nc.vector.memset(bias_sb[:, 0:NXT], 0.0)
        ywT_sb = pers.tile([128, M // 128], F32, tag="ywT")
        nc.sync.dma_start(ywT_sb[:], ywT)
        ywc = pers.tile([128, M // 128], F32, tag="ywc")
        nc.vector.tensor_scalar_max(ywc[:], ywT_sb[:], 1e-4)
        nc.scalar.activation(bias_sb[:, NXT:KT], ywc[:], AF.Ln, bias=z128[:])

        # proj bias broadcast (gpsimd is idle at start)
        bpr_sb = pers.tile([1, C], BF16, tag="bpr")
        nc.sync.dma_start(bpr_sb[:], bpr)
        bb_sb = pers.tile([128, C], BF16, tag="bb")
        nc.gpsimd.partition_broadcast(bb_sb[:], bpr_sb[:])

        cs_sb = pers.tile([RD, N], BF16, tag="cs")
        nc.sync.dma_start(cs_sb[:], cs)
        sn_sb = pers.tile([RD, N], BF16, tag="sn")
        nc.sync.dma_start(sn_sb[:], sn)

        x_sb = pers.tile([128, NCT, N], BF16, tag="x")
        for xc in range(4):
            nc.sync.dma_start(
                x_sb[:, :, xc * 512 : (xc + 1) * 512],
                xT[:, xc * 512 : (xc + 1) * 512].rearrange(
                    "(ct p) n -> p ct n", p=128
                ),
            )
        y_sb = pers.tile([128, NCT, M], BF16, tag="y")
        nc.sync.dma_start(y_sb[:], yT.rearrange("(ct p) n -> p ct n", p=128))

        v_sb = pers.tile([128, KT, NHL * HD], BF16, tag="v")

        # pools
        knp = ctx.enter_context(tc.tile_pool(name="knp", bufs=2))
        qnp = ctx.enter_context(tc.tile_pool(name="qnp", bufs=2))
        wqkp = ctx.enter_context(tc.tile_pool(name="wqkp", bufs=2))
        wkyp = ctx.enter_context(tc.tile_pool(name="wkyp", bufs=2))
        sqp = ctx.enter_context(tc.tile_pool(name="sqp", bufs=2))
        rwp = ctx.enter_context(tc.tile_pool(name="rwp", bufs=1))
        binvp = ctx.enter_context(tc.tile_pool(name="binvp", bufs=2))
        ropep = ctx.enter_context(tc.tile_pool(name="ropep", bufs=2))
        exp_ = ctx.enter_context(tc.tile_pool(name="exp", bufs=4))
        exsp = ctx.enter_context(tc.tile_pool(name="exsp", bufs=3))
        invdp = ctx.enter_context(tc.tile_pool(name="invdp", bufs=2))
        bdenp = ctx.enter_context(tc.tile_pool(name="bdenp", bufs=2))
        avsp = ctx.enter_context(tc.tile_pool(name="avsp", bufs=2))
        op = ctx.enter_context(tc.tile_pool(name="op", bufs=2))
        late = {}

        psbig = ctx.enter_context(tc.tile_pool(name="psbig", bufs=2, space="PSUM"))
        psSc = ctx.enter_context(tc.tile_pool(name="psSc", bufs=2, space="PSUM"))
        psAv = ctx.enter_context(tc.tile_pool(name="psAv", bufs=1, space="PSUM"))
        psSm = ctx.enter_context(tc.tile_pool(name="psSm", bufs=2, space="PSUM"))
        psDen = ctx.enter_context(tc.tile_pool(name="psDen", bufs=1, space="PSUM"))

        a2a_ins = [
            dram.tile([2 * NQC, 128, 512], BF16, name=f"a2ai{t}") for t in range(NHL)
        ]
        a2a_outs = [
            dram.tile([2 * NQC, 128, 512], BF16, name=f"a2ao{t}") for t in range(NHL)
        ]

        kn = [None] * NHL
        qn = [None] * NHL
        wp_t = [None] * NHL
        pj_t = [None] * NHL
        acc = [None] * 12

        def emit_vtile(kt):
            """v for all 3 heads, kv tile kt -> v_sb[:, kt, :]."""
            ps = psbig.tile([128, 512], F32, tag="big", name=f"vps{kt}")[:, : NHL * HD]
            if kt < NXT:
                src, w, t0 = x_sb, wvx_sb, kt * 128
            else:
                src, w, t0 = y_sb, wvy_sb, (kt - NXT) * 128
            for ct in range(NCT):
                nc.tensor.matmul(
                    ps[:],
                    src[:, ct, t0 : t0 + 128],
                    w[:, ct, :],
                    start=(ct == 0),
                    stop=(ct == NCT - 1),
                )
            nc.vector.tensor_copy(v_sb[:, kt, :], ps[:])

        def rope(dst, q0):
            """in-place rope on dst [128, CH] (first RD partitions)."""
            hf = RD // 2
            csc = cs_sb[:, q0 : q0 + CH]
            snc = sn_sb[:, q0 : q0 + CH]
            sw = ropep.tile([RD, CH], BF16, tag="sw", name="sw")
            nc.vector.tensor_copy(sw[0:hf, :], dst[hf:RD, :])
            nc.vector.tensor_copy(sw[hf:RD, :], dst[0:hf, :])
            ma = ropep.tile([RD, CH], BF16, tag="ma", name="ma")
            nc.vector.tensor_mul(ma[:], dst[0:RD, :], csc)
            mb = ropep.tile([RD, CH], BF16, tag="mb", name="mb")
            nc.vector.tensor_mul(mb[:], sw[:], snc)
            nc.vector.tensor_add(dst[0:RD, :], ma[:], mb[:])

        def head_pieces(t):
            """Zero-arg emission closures for head t's q/k projections+norms."""
            pieces = []
            kn[t] = knp.tile([128, S], BF16, tag="kn", name=f"kn{t}")
            qn[t] = qnp.tile([128, N], BF16, tag="qn", name=f"qn{t}")
            wqk_sb = wqkp.tile([128, NCT, 256], BF16, tag="wqk", name=f"wqk{t}")
            wky_sb = wkyp.tile([128, NCT, 128], BF16, tag="wky", name=f"wky{t}")
            st = {}

            def load_w():
                nc.sync.dma_start(
                    wqk_sb[:],
                    wqk[:, t * 256 : (t + 1) * 256].rearrange(
                        "(ct p) c -> p ct c", p=128
                    ),
                )
                nc.sync.dma_start(
                    wky_sb[:],
                    wky[:, t * 128 : (t + 1) * 128].rearrange(
                        "(ct p) c -> p ct c", p=128
                    ),
                )
                st["ri"] = rwp.tile([1, ROWW], BF16, tag="ri", name=f"ri{t}")
            pieces.append(load_w)

            # chunk c: 0..3 = k from x, 4 = k from y, 5..8 = q from x
            def chunk(c):
                is_q = c >= NKC
                if is_q:
                    src, w_sb, wof = x_sb, wqk_sb, 0
                    q0 = (c - NKC) * CH
                    dst, doff = qn[t], q0
                    rowoff = S + q0
                elif c == NKC - 1:
                    src, w_sb, wof = y_sb, wky_sb, 0
                    q0 = 0
                    dst, doff = kn[t], N
                    rowoff = N
                else:
                    src, w_sb, wof = x_sb, wqk_sb, 128
                    q0 = c * CH
                    dst, doff = kn[t], q0
                    rowoff = q0

                def mm():
                    ps = psbig.tile([128, 512], F32, tag="big", name=f"ps{t}_{c}")
                    st[c] = ps
                    for ct in range(NCT):
                        nc.tensor.matmul(
                            ps[:],
                            w_sb[:, ct, wof : wof + 128],
                            src[:, ct, q0 : q0 + CH],
                            start=(ct == 0),
                            stop=(ct == NCT - 1),
                        )
                pieces.append(mm)

                def post():
                    ps = st[c]
                    sq = sqp.tile([128, CH], BF16, tag="sq", name=f"sq{t}_{c}")
                    nc.scalar.activation(sq[:], ps[:], AF.Square, bias=z128[:])
                    nc.vector.tensor_copy(dst[:, doff : doff + CH], ps[:])
                    pr = psSm.tile([1, CH], F32, tag="sm", name=f"pr{t}_{c}")
                    nc.tensor.matmul(
                        pr[:], onesb_sb[:], sq[:], start=True, stop=True
                    )
                    nc.vector.tensor_copy(
                        st["ri"][0:1, rowoff : rowoff + CH], pr[:]
                    )
                pieces.append(post)

                def finish():
                    binv = binvp.tile([128, CH], BF16, tag="binv", name=f"bv{t}_{c}")
                    nc.gpsimd.partition_broadcast(
                        binv[:], st["ri"][0:1, rowoff : rowoff + CH]
                    )
                    d = dst[:, doff : doff + CH]
                    if is_q:
                        nc.vector.scalar_tensor_tensor(
                            d, d, hdm_sb[:], binv[:], op0=ALU.mult, op1=ALU.mult
                        )
                    else:
                        nc.vector.tensor_mul(d, d, binv[:])
                    if not (not is_q and c == NKC - 1):
                        rope(d, q0)
                return finish

            finishes = [chunk(c) for c in range(NKC + NQC)]

            def inv_piece():
                # one table switch in, one out, per head
                nc.scalar.activation(
                    st["ri"][0:1, :], st["ri"][0:1, :], AF.Abs_reciprocal_sqrt,
                    scale=1.0 / HD, bias=eps1[:],
                )
            pieces.append(inv_piece)
            pieces.extend(finishes)
            return pieces

        def emit_attention(t, fillers, extra_per_qc=None, vzip=False):
            fi = [0]

            def fill(n):
                while n > 0 and fi[0] < len(fillers):
                    fillers[fi[0]]()
                    fi[0] += 1
                    n -= 1

            for qc in range(NQC):
                av = psAv.tile([128, 512], F32, tag="av")
                den = psDen.tile([1, 512], F32, tag="den")
                pair = None
                npair = 0
                for kt in range(KT):
                    if vzip and qc == 0 and 3 <= kt <= 18:
                        emit_vtile(kt + 1)
                    sc = psSc.tile([128, 512], F32, tag="sc")
                    nc.tensor.matmul(
                        sc[:],
                        kn[t][:, kt * 128 : (kt + 1) * 128],
                        qn[t][:, qc * 512 : (qc + 1) * 512],
                        start=True,
                        stop=True,
                    )
                    ex = exp_.tile([128, 512], BF16, tag="ex")
                    nc.scalar.activation(
                        ex[:], sc[:], AF.Exp, bias=bias_sb[:, kt : kt + 1]
                    )
                    nc.tensor.matmul(
                        av[:],
                        v_sb[:, kt, t * HD : (t + 1) * HD],
                        ex[:],
                        start=(kt == 0),
                        stop=(kt == KT - 1),
                    )
                    if pair is None:
                        pair = ex
                    else:
                        exs = exsp.tile([128, 512], BF16, tag="exs")
                        nc.vector.tensor_add(exs[:], pair[:], ex[:])
                        nc.tensor.matmul(
                            den[:],
                            onesb_sb[:],
                            exs[:],
                            start=(npair == 0),
                            stop=(npair == KT // 2 - 1),
                        )
                        npair += 1
                        pair = None
                    if vzip and qc == 0:
                        if kt % 4 == 2:
                            fill(1)
                    elif kt % 2 == 1:
                        fill(1)
                invd = invdp.tile([1, 512], F32, tag="invd")
                nc.vector.reciprocal(invd[:], den[:])
                bden = bdenp.tile([128, 512], F32, tag="bden")
                nc.gpsimd.partition_broadcast(bden[:], invd[:])
                avs = avsp.tile([128, 512], F32, tag="avs")
                nc.vector.tensor_copy(avs[:], av[:])
                o0 = op.tile([128, 512], BF16, tag="o0")
                nc.vector.scalar_tensor_tensor(
                    o0[:], avs[:], mska_sb[:], bden[:], op0=ALU.mult, op1=ALU.mult
                )
                o1 = op.tile([128, 512], BF16, tag="o1")
                nc.vector.scalar_tensor_tensor(
                    o1[:], avs[:], mskb_sb[:], bden[:], op0=ALU.mult, op1=ALU.mult
                )
                nc.sync.dma_start(a2a_ins[t][qc], o0[:])
                nc.sync.dma_start(a2a_ins[t][NQC + qc], o1[:])
                if extra_per_qc is not None:
                    extra_per_qc(qc)
                fill(2)
            fill(len(fillers))

        def emit_a2a(t):
            nc.gpsimd.collective_compute(
                "AllToAll",
                ALU.bypass,
                replica_groups=[[0, 1, 2, 3, 4, 5, 6, 7]],
                ins=[a2a_ins[t].opt()],
                outs=[a2a_outs[t].opt()],
            )

        def emit_wp_load(t):
            wp_t[t] = late["wpp"].tile([128, NCT, 512], BF16, tag="wp", name=f"wp{t}")
            for i in range(4):
                nc.sync.dma_start(
                    wp_t[t][:, 3 * i : 3 * (i + 1), :],
                    wproj[t * 512 + i * 128 : t * 512 + (i + 1) * 128, :],
                )

        def emit_pj_load(t):
            pj_t[t] = late["pjp"].tile(
                [128, 2 * NQC, 512], BF16, tag="pj", name=f"pj{t}"
            )
            for i in range(2 * NQC):
                nc.sync.dma_start(pj_t[t][:, i, :], a2a_outs[t][i])

        def emit_proj(t):
            pj = pj_t[t]
            pjs = late["pjsp"].tile([128, 4, 512], BF16, tag="pjs", name=f"pjs{t}")
            for i in range(4):
                nc.vector.tensor_add(pjs[:, i, :], pj[:, i, :], pj[:, 4 + i, :])
            for fc in range(3):
                for tcc in range(4):
                    pp = psbig.tile([128, 512], F32, tag="big", name=f"pp{t}")
                    for i in range(4):
                        nc.tensor.matmul(
                            pp[:],
                            pjs[:, i, tcc * 128 : (tcc + 1) * 128],
                            wp_t[t][:, 3 * i + fc, :],
                            start=(i == 0),
                            stop=(i == 3),
                        )
                    if t == 0:
                        acc[fc * 4 + tcc] = late["accp"].tile(
                            [128, 512], BF16, tag=f"acc{fc * 4 + tcc}",
                            name=f"acc{fc * 4 + tcc}",
                        )
                        nc.vector.tensor_copy(acc[fc * 4 + tcc][:], pp[:])
                    elif t == 1:
                        a = acc[fc * 4 + tcc]
                        nc.vector.tensor_add(a[:], a[:], pp[:])
                    else:
                        a = acc[fc * 4 + tcc]
                        nc.vector.tensor_add(a[:], a[:], pp[:])
                        ob = avsp.tile(
                            [128, 512], F32, tag="avs", name=f"ob{fc}_{tcc}"
                        )
                        nc.vector.tensor_tensor(
                            ob[:], a[:], bb_sb[:, fc * 512 : (fc + 1) * 512],
                            ALU.add,
                        )
                        nc.sync.dma_start(
                            out[
                                tcc * 128 : (tcc + 1) * 128,
                                fc * 512 : (fc + 1) * 512,
                            ],
                            ob[:],
                        )

        # ---------------- emission schedule ----------------
        with tc.tile_pool(name="wvp", bufs=1) as wvp:
            wvx_sb = wvp.tile([128, NCT, NHL * HD], BF16, tag="wvx")
            nc.sync.dma_start(wvx_sb[:], wvx.rearrange("(ct p) c -> p ct c", p=128))
            wvy_sb = wvp.tile([128, NCT, NHL * HD], BF16, tag="wvy")
            nc.sync.dma_start(wvy_sb[:], wvy.rearrange("(ct p) c -> p ct c", p=128))

            for p in head_pieces(0):
                p()
            for kt in range(4):
                emit_vtile(kt)

            fi1 = head_pieces(1)
            emit_attention(0, fi1, vzip=True)
            emit_a2a(0)
        late["wpp"] = ctx.enter_context(tc.tile_pool(name="wpp", bufs=1))
        late["pjp"] = ctx.enter_context(tc.tile_pool(name="pjp", bufs=1))
        late["pjsp"] = ctx.enter_context(tc.tile_pool(name="pjsp", bufs=1))
        late["accp"] = ctx.enter_context(tc.tile_pool(name="accp", bufs=1))
        emit_wp_load(0)

        fi2 = head_pieces(2)

        def extra1(qc):
            if qc == 0:
                emit_pj_load(0)
            elif qc == 1:
                emit_proj(0)
                emit_wp_load(1)

        emit_attention(1, fi2, extra_per_qc=extra1)
        emit_a2a(1)

        def extra2(qc):
            if qc == 0:
                emit_pj_load(1)
            elif qc == 1:
                emit_proj(1)
                emit_wp_load(2)

        emit_attention(2, [], extra_per_qc=extra2)
        emit_a2a(2)
        emit_pj_load(2)
        emit_proj(2)
    nc.compile()
    return nc


_NC_CACHE = {}


def _get_nc():
    if "nc" not in _NC_CACHE:
        _NC_CACHE["nc"] = build_nc()
    return _NC_CACHE["nc"]


def make_in_maps(x, y, pos, y_token_weights, Wqkv, Wkv, q_norm_w, k_norm_w, Wproj, bproj):
    f = np.float32
    bf = ml_dtypes.bfloat16
    pos = np.asarray(pos, dtype=f)
    c32 = pos[:, :, 0].T
    s32 = pos[:, :, 1].T
    csT = np.ascontiguousarray(np.concatenate([c32, c32], 0).astype(bf))   # [64, N]
    snT = np.ascontiguousarray(np.concatenate([-s32, s32], 0).astype(bf))  # [64, N]
    wq = np.asarray(q_norm_w, dtype=f)
    wk = np.asarray(k_norm_w, dtype=f)
    Wqkv = np.asarray(Wqkv, dtype=f)
    Wkv = np.asarray(Wkv, dtype=f)
    Wp = np.asarray(Wproj, dtype=f)

    wproj_rows = np.empty((NHL, 4, 128, C), dtype=f)
    for t in range(NHL):
        for i in range(4):
            h = 3 * i + t
            wproj_rows[t, i] = Wp[h * 128 : (h + 1) * 128, :]
    wproj_host = np.ascontiguousarray(wproj_rows.reshape(NHL * 4 * 128, C).astype(bf))

    in_maps = []
    for c in range(8):
        b, g = c // 4, c % 4
        heads = [3 * g + i for i in range(NHL)]
        qk_cols = []
        ky_cols = []
        vx_cols = []
        vy_cols = []
        for h in heads:
            qk_cols.append(Wqkv[:, h * HD : (h + 1) * HD] * wq[None, :])
            qk_cols.append(Wqkv[:, C + h * HD : C + (h + 1) * HD] * wk[None, :])
            ky_cols.append(Wkv[:, h * HD : (h + 1) * HD] * wk[None, :])
            vx_cols.append(Wqkv[:, 2 * C + h * HD : 2 * C + (h + 1) * HD])
            vy_cols.append(Wkv[:, C + h * HD : C + (h + 1) * HD])
        in_maps.append(
            {
                "xT": np.ascontiguousarray(np.asarray(x)[b].T.astype(bf)),
                "yT": np.ascontiguousarray(np.asarray(y)[b].T.astype(bf)),
                "wqk": np.ascontiguousarray(np.concatenate(qk_cols, axis=1).astype(bf)),
                "wky": np.ascontiguousarray(np.concatenate(ky_cols, axis=1).astype(bf)),
                "wvx": np.ascontiguousarray(np.concatenate(vx_cols, axis=1).astype(bf)),
                "wvy": np.ascontiguousarray(np.concatenate(vy_cols, axis=1).astype(bf)),
                "wproj": wproj_host,
                "cs": csT,
                "sn": snT,
                "ywT": np.ascontiguousarray(
                    np.asarray(y_token_weights)[b].reshape(M // 128, 128).T, dtype=f
                ),
                "bpr": np.asarray(bproj, dtype=f).reshape(1, C).astype(bf),
                "onesb": np.ones((128, 1), dtype=bf),
                "mska": np.full((128, 1), 1.0 if b == 0 else 0.0, dtype=f),
                "mskb": np.full((128, 1), 0.0 if b == 0 else 1.0, dtype=f),
            }
        )
    return in_maps


def kernel(x, y, pos, y_token_weights, Wqkv, Wkv, q_norm_w, k_norm_w, Wproj, bproj,
           _trace=False):
    x = np.asarray(x, dtype=np.float32)
    y = np.asarray(y, dtype=np.float32)
    pos = np.asarray(pos, dtype=np.float32)
    y_token_weights = np.asarray(y_token_weights, dtype=np.float32)
    nc = _get_nc()
    in_maps = make_in_maps(
        x, y, pos, y_token_weights,
        np.asarray(Wqkv), np.asarray(Wkv), np.asarray(q_norm_w),
        np.asarray(k_norm_w), np.asarray(Wproj), np.asarray(bproj),
    )
    res = run_bass_kernel_spmd(nc, in_maps, core_ids=list(range(8)), trace=_trace)
    outp = np.zeros((B, N, C), dtype=np.float32)
    for c in range(8):
        b, g = c // 4, c % 4
        outp[b, g * 512 : (g + 1) * 512, :] = res.results[c]["out"]
    if _trace:
        return outp, res
    return outp


# revision 16
# speedup vs baseline: 1.1351x; 1.1351x over previous
"""Distributed Bass kernel for nn_Attention_12953621365048 (8 TRN2 NeuronCores).

Sharding: 2 batch-groups x 4 head-groups (3 heads/core); core c handles
batch b=c//4, heads 3*(c%4)..3*(c%4)+2.

Fused per-head pipeline: the QKV projection + norms of head t+1 are
emitted as "filler" pieces inside the ACT-bound attention stream of
head t so the PE never idles; the V projection for all heads is zipped
per-kv-tile into attention(head 0)'s first q-chunk.

RMSNorm: per-chunk sum-of-squares rows are packed into one [1, 4608]
SBUF row (partition 0) at free offsets, so a single Abs_reciprocal_sqrt
per head computes every inv (2 activation-table loads per head, between
the exp streams); normalization is applied in place on kn/qn before an
in-place rope whose swap copies ride the DVE (partition-shifted copies
are legal, shifted arithmetic is not).

A2A: each core writes its attention output twice (blocks qc and 4+qc),
masked by per-core 0/1 scalars (batch 0 cores zero the upper blocks,
batch 1 cores the lower) folded into the output-normalization multiply.
Every destination sums block pairs j/(4+j) -- one real, one zero --
giving a uniform program with a 4-block (unpadded) projection.
"""

import math
from contextlib import ExitStack

import numpy as np
import ml_dtypes

import concourse.mybir as mybir
import concourse.tile as tile
from concourse import bacc
from concourse.bass_utils import run_bass_kernel_spmd

B, N, M, C, H, HD, RD = 2, 2048, 512, 1536, 12, 128, 64
EPS = 1e-6
NHL = 3               # heads per core
S = N + M             # 2560 kv tokens
KT = S // 128         # 20 kv tiles
NXT = N // 128        # 16 x kv tiles
NQC = N // 512        # 4 q-chunks of 512
CH = 512              # qkv-phase token chunk
NKC = 5               # k chunks (4 x + 1 y)
NCT = C // 128        # 12 contraction tiles
ROWW = S + N          # 4608: k rows at [0,S), q rows at [S, S+N)
F32 = mybir.dt.float32
BF16 = mybir.dt.bfloat16
AF = mybir.ActivationFunctionType
ALU = mybir.AluOpType


def build_nc():
    nc = bacc.Bacc("TRN2", target_bir_lowering=False, debug=False, num_devices=8)

    xT = nc.dram_tensor("xT", [C, N], BF16, kind="ExternalInput").ap()
    yT = nc.dram_tensor("yT", [C, M], BF16, kind="ExternalInput").ap()
    wqk = nc.dram_tensor("wqk", [C, NHL * 256], BF16, kind="ExternalInput").ap()
    wky = nc.dram_tensor("wky", [C, NHL * 128], BF16, kind="ExternalInput").ap()
    wvx = nc.dram_tensor("wvx", [C, NHL * HD], BF16, kind="ExternalInput").ap()
    wvy = nc.dram_tensor("wvy", [C, NHL * HD], BF16, kind="ExternalInput").ap()
    # proj rows grouped [t][i] = Wproj rows of global head 3i+t (batch indep.)
    wproj = nc.dram_tensor("wproj", [NHL * 4 * 128, C], BF16, kind="ExternalInput").ap()
    cs = nc.dram_tensor("cs", [RD, N], BF16, kind="ExternalInput").ap()
    sn = nc.dram_tensor("sn", [RD, N], BF16, kind="ExternalInput").ap()
    ywT = nc.dram_tensor("ywT", [128, M // 128], F32, kind="ExternalInput").ap()
    bpr = nc.dram_tensor("bpr", [1, C], BF16, kind="ExternalInput").ap()
    onesb = nc.dram_tensor("onesb", [128, 1], BF16, kind="ExternalInput").ap()
    mska = nc.dram_tensor("mska", [128, 1], F32, kind="ExternalInput").ap()
    mskb = nc.dram_tensor("mskb", [128, 1], F32, kind="ExternalInput").ap()
    out = nc.dram_tensor("out", [512, C], F32, kind="ExternalOutput").ap()

    with tile.TileContext(nc) as tc, ExitStack() as ctx:
        pers = ctx.enter_context(tc.tile_pool(name="persist", bufs=1))
        dram = ctx.enter_context(tc.tile_pool(name="dram", bufs=1, space="DRAM"))

        onesb_sb = pers.tile([128, 1], BF16, tag="onesb")
        nc.sync.dma_start(onesb_sb[:], onesb)
        mska_sb = pers.tile([128, 1], F32, tag="mska")
        nc.sync.dma_start(mska_sb[:], mska)
        mskb_sb = pers.tile([128, 1], F32, tag="mskb")
        nc.sync.dma_start(mskb_sb[:], mskb)
        hdm_sb = pers.tile([128, 1], F32, tag="hdm")
        nc.vector.memset(hdm_sb[:], HD ** -0.5)
        z128 = pers.tile([128, 1], F32, tag="z128")
        nc.vector.memset(z128[:], 0.0)
        eps1 = pers.tile([1, 1], F32, tag="eps1")
        nc.vector.memset(eps1[:], EPS)

        # attention bias per kv tile column: 0 for x tiles, ln(clip(w)) for y
        bias_sb = pers.tile([128, KT], F32, tag="bias")
        nc.vector.memset(bias_sb[:, 0:NXT], 0.0)
        ywT_sb = pers.tile([128, M // 128], F32, tag="ywT")
        nc.sync.dma_start(ywT_sb[:], ywT)
        ywc = pers.tile([128, M // 128], F32, tag="ywc")
        nc.vector.tensor_scalar_max(ywc[:], ywT_sb[:], 1e-4)
        nc.scalar.activation(bias_sb[:, NXT:KT], ywc[:], AF.Ln, bias=z128[:])

        # proj bias broadcast (gpsimd is idle at start)
        bpr_sb = pers.tile([1, C], BF16, tag="bpr")
        nc.sync.dma_start(bpr_sb[:], bpr)
        bb_sb = pers.tile([128, C], BF16, tag="bb")
        nc.gpsimd.partition_broadcast(bb_sb[:], bpr_sb[:])

        cs_sb = pers.tile([RD, N], BF16, tag="cs")
        nc.sync.dma_start(cs_sb[:], cs)
        sn_sb = pers.tile([RD, N], BF16, tag="sn")
        nc.sync.dma_start(sn_sb[:], sn)

        x_sb = pers.tile([128, NCT, N], BF16, tag="x")
        nc.sync.dma_start(x_sb[:], xT.rearrange("(ct p) n -> p ct n", p=128))
        y_sb = pers.tile([128, NCT, M], BF16, tag="y")
        nc.sync.dma_start(y_sb[:], yT.rearrange("(ct p) n -> p ct n", p=128))

        v_sb = pers.tile([128, KT, NHL * HD], BF16, tag="v")

        # pools
        knp = ctx.enter_context(tc.tile_pool(name="knp", bufs=2))
        qnp = ctx.enter_context(tc.tile_pool(name="qnp", bufs=2))
        wqkp = ctx.enter_context(tc.tile_pool(name="wqkp", bufs=2))
        wkyp = ctx.enter_context(tc.tile_pool(name="wkyp", bufs=2))
        sqp = ctx.enter_context(tc.tile_pool(name="sqp", bufs=2))
        rwp = ctx.enter_context(tc.tile_pool(name="rwp", bufs=1))
        binvp = ctx.enter_context(tc.tile_pool(name="binvp", bufs=2))
        ropep = ctx.enter_context(tc.tile_pool(name="ropep", bufs=2))
        exp_ = ctx.enter_context(tc.tile_pool(name="exp", bufs=4))
        exsp = ctx.enter_context(tc.tile_pool(name="exsp", bufs=3))
        invdp = ctx.enter_context(tc.tile_pool(name="invdp", bufs=2))
        bdenp = ctx.enter_context(tc.tile_pool(name="bdenp", bufs=2))
        avsp = ctx.enter_context(tc.tile_pool(name="avsp", bufs=2))
        op = ctx.enter_context(tc.tile_pool(name="op", bufs=2))
        late = {}

        psbig = ctx.enter_context(tc.tile_pool(name="psbig", bufs=2, space="PSUM"))
        psSc = ctx.enter_context(tc.tile_pool(name="psSc", bufs=2, space="PSUM"))
        psAv = ctx.enter_context(tc.tile_pool(name="psAv", bufs=1, space="PSUM"))
        psSm = ctx.enter_context(tc.tile_pool(name="psSm", bufs=2, space="PSUM"))
        psDen = ctx.enter_context(tc.tile_pool(name="psDen", bufs=1, space="PSUM"))

        a2a_ins = [
            dram.tile([2 * NQC, 128, 512], BF16, name=f"a2ai{t}") for t in range(NHL)
        ]
        a2a_outs = [
            dram.tile([2 * NQC, 128, 512], BF16, name=f"a2ao{t}") for t in range(NHL)
        ]

        kn = [None] * NHL
        qn = [None] * NHL
        wp_t = [None] * NHL
        pj_t = [None] * NHL
        acc = [None] * 12

        def emit_vtile(kt):
            """v for all 3 heads, kv tile kt -> v_sb[:, kt, :]."""
            ps = psbig.tile([128, 512], F32, tag="big", name=f"vps{kt}")[:, : NHL * HD]
            if kt < NXT:
                src, w, t0 = x_sb, wvx_sb, kt * 128
            else:
                src, w, t0 = y_sb, wvy_sb, (kt - NXT) * 128
            for ct in range(NCT):
                nc.tensor.matmul(
                    ps[:],
                    src[:, ct, t0 : t0 + 128],
                    w[:, ct, :],
                    start=(ct == 0),
                    stop=(ct == NCT - 1),
                )
            nc.vector.tensor_copy(v_sb[:, kt, :], ps[:])

        def rope(dst, q0):
            """in-place rope on dst [128, CH] (first RD partitions)."""
            hf = RD // 2
            csc = cs_sb[:, q0 : q0 + CH]
            snc = sn_sb[:, q0 : q0 + CH]
            sw = ropep.tile([RD, CH], BF16, tag="sw", name="sw")
            nc.vector.tensor_copy(sw[0:hf, :], dst[hf:RD, :])
            nc.vector.tensor_copy(sw[hf:RD, :], dst[0:hf, :])
            ma = ropep.tile([RD, CH], BF16, tag="ma", name="ma")
            nc.vector.tensor_mul(ma[:], dst[0:RD, :], csc)
            mb = ropep.tile([RD, CH], BF16, tag="mb", name="mb")
            nc.vector.tensor_mul(mb[:], sw[:], snc)
            nc.vector.tensor_add(dst[0:RD, :], ma[:], mb[:])

        def head_pieces(t):
            """Zero-arg emission closures for head t's q/k projections+norms."""
            pieces = []
            kn[t] = knp.tile([128, S], BF16, tag="kn", name=f"kn{t}")
            qn[t] = qnp.tile([128, N], BF16, tag="qn", name=f"qn{t}")
            wqk_sb = wqkp.tile([128, NCT, 256], BF16, tag="wqk", name=f"wqk{t}")
            wky_sb = wkyp.tile([128, NCT, 128], BF16, tag="wky", name=f"wky{t}")
            st = {}

            def load_w():
                nc.sync.dma_start(
                    wqk_sb[:],
                    wqk[:, t * 256 : (t + 1) * 256].rearrange(
                        "(ct p) c -> p ct c", p=128
                    ),
                )
                nc.sync.dma_start(
                    wky_sb[:],
                    wky[:, t * 128 : (t + 1) * 128].rearrange(
                        "(ct p) c -> p ct c", p=128
                    ),
                )
                st["ri"] = rwp.tile([1, ROWW], BF16, tag="ri", name=f"ri{t}")
            pieces.append(load_w)

            # chunk c: 0..3 = k from x, 4 = k from y, 5..8 = q from x
            def chunk(c):
                is_q = c >= NKC
                if is_q:
                    src, w_sb, wof = x_sb, wqk_sb, 0
                    q0 = (c - NKC) * CH
                    dst, doff = qn[t], q0
                    rowoff = S + q0
                elif c == NKC - 1:
                    src, w_sb, wof = y_sb, wky_sb, 0
                    q0 = 0
                    dst, doff = kn[t], N
                    rowoff = N
                else:
                    src, w_sb, wof = x_sb, wqk_sb, 128
                    q0 = c * CH
                    dst, doff = kn[t], q0
                    rowoff = q0

                def mm():
                    ps = psbig.tile([128, 512], F32, tag="big", name=f"ps{t}_{c}")
                    st[c] = ps
                    for ct in range(NCT):
                        nc.tensor.matmul(
                            ps[:],
                            w_sb[:, ct, wof : wof + 128],
                            src[:, ct, q0 : q0 + CH],
                            start=(ct == 0),
                            stop=(ct == NCT - 1),
                        )
                pieces.append(mm)

                def post():
                    ps = st[c]
                    sq = sqp.tile([128, CH], BF16, tag="sq", name=f"sq{t}_{c}")
                    nc.scalar.activation(sq[:], ps[:], AF.Square, bias=z128[:])
                    nc.vector.tensor_copy(dst[:, doff : doff + CH], ps[:])
                    pr = psSm.tile([1, CH], F32, tag="sm", name=f"pr{t}_{c}")
                    nc.tensor.matmul(
                        pr[:], onesb_sb[:], sq[:], start=True, stop=True
                    )
                    nc.vector.tensor_copy(
                        st["ri"][0:1, rowoff : rowoff + CH], pr[:]
                    )
                pieces.append(post)

                def finish():
                    binv = binvp.tile([128, CH], BF16, tag="binv", name=f"bv{t}_{c}")
                    nc.gpsimd.partition_broadcast(
                        binv[:], st["ri"][0:1, rowoff : rowoff + CH]
                    )
                    d = dst[:, doff : doff + CH]
                    if is_q:
                        nc.vector.scalar_tensor_tensor(
                            d, d, hdm_sb[:], binv[:], op0=ALU.mult, op1=ALU.mult
                        )
                    else:
                        nc.vector.tensor_mul(d, d, binv[:])
                    if not (not is_q and c == NKC - 1):
                        rope(d, q0)
                return finish

            finishes = [chunk(c) for c in range(NKC + NQC)]

            def inv_piece():
                # one table switch in, one out, per head
                nc.scalar.activation(
                    st["ri"][0:1, :], st["ri"][0:1, :], AF.Abs_reciprocal_sqrt,
                    scale=1.0 / HD, bias=eps1[:],
                )
            pieces.append(inv_piece)
            pieces.extend(finishes)
            return pieces

        def emit_attention(t, fillers, extra_per_qc=None, vzip=False):
            fi = [0]

            def fill(n):
                while n > 0 and fi[0] < len(fillers):
                    fillers[fi[0]]()
                    fi[0] += 1
                    n -= 1

            for qc in range(NQC):
                av = psAv.tile([128, 512], F32, tag="av")
                den = psDen.tile([1, 512], F32, tag="den")
                pair = None
                npair = 0
                for kt in range(KT):
                    if vzip and qc == 0 and 3 <= kt <= 18:
                        emit_vtile(kt + 1)
                    sc = psSc.tile([128, 512], F32, tag="sc")
                    nc.tensor.matmul(
                        sc[:],
                        kn[t][:, kt * 128 : (kt + 1) * 128],
                        qn[t][:, qc * 512 : (qc + 1) * 512],
                        start=True,
                        stop=True,
                    )
                    ex = exp_.tile([128, 512], BF16, tag="ex")
                    nc.scalar.activation(
                        ex[:], sc[:], AF.Exp, bias=bias_sb[:, kt : kt + 1]
                    )
                    nc.tensor.matmul(
                        av[:],
                        v_sb[:, kt, t * HD : (t + 1) * HD],
                        ex[:],
                        start=(kt == 0),
                        stop=(kt == KT - 1),
                    )
                    if pair is None:
                        pair = ex
                    else:
                        exs = exsp.tile([128, 512], BF16, tag="exs")
                        nc.vector.tensor_add(exs[:], pair[:], ex[:])
                        nc.tensor.matmul(
                            den[:],
                            onesb_sb[:],
                            exs[:],
                            start=(npair == 0),
                            stop=(npair == KT // 2 - 1),
                        )
                        npair += 1
                        pair = None
                    if not (vzip and qc == 0) and kt in (2, 5, 8, 11, 14, 17):
                        fill(1)
                invd = invdp.tile([1, 512], F32, tag="invd")
                nc.vector.reciprocal(invd[:], den[:])
                bden = bdenp.tile([128, 512], F32, tag="bden")
                nc.gpsimd.partition_broadcast(bden[:], invd[:])
                avs = avsp.tile([128, 512], F32, tag="avs")
                nc.vector.tensor_copy(avs[:], av[:])
                o0 = op.tile([128, 512], BF16, tag="o0")
                nc.vector.scalar_tensor_tensor(
                    o0[:], avs[:], mska_sb[:], bden[:], op0=ALU.mult, op1=ALU.mult
                )
                o1 = op.tile([128, 512], BF16, tag="o1")
                nc.vector.scalar_tensor_tensor(
                    o1[:], avs[:], mskb_sb[:], bden[:], op0=ALU.mult, op1=ALU.mult
                )
                nc.sync.dma_start(a2a_ins[t][qc], o0[:])
                nc.sync.dma_start(a2a_ins[t][NQC + qc], o1[:])
                if extra_per_qc is not None:
                    extra_per_qc(qc)
                fill(2)
            fill(len(fillers))

        def emit_a2a(t):
            nc.gpsimd.collective_compute(
                "AllToAll",
                ALU.bypass,
                replica_groups=[[0, 1, 2, 3, 4, 5, 6, 7]],
                ins=[a2a_ins[t].opt()],
                outs=[a2a_outs[t].opt()],
            )

        def emit_wp_load(t):
            wp_t[t] = late["wpp"].tile([128, NCT, 512], BF16, tag="wp", name=f"wp{t}")
            for i in range(4):
                nc.sync.dma_start(
                    wp_t[t][:, 3 * i : 3 * (i + 1), :],
                    wproj[t * 512 + i * 128 : t * 512 + (i + 1) * 128, :],
                )

        def emit_pj_load(t):
            pj_t[t] = late["pjp"].tile(
                [128, 2 * NQC, 512], BF16, tag="pj", name=f"pj{t}"
            )
            for i in range(2 * NQC):
                nc.sync.dma_start(pj_t[t][:, i, :], a2a_outs[t][i])

        def emit_proj(t):
            pj = pj_t[t]
            pjs = late["pjsp"].tile([128, 4, 512], BF16, tag="pjs", name=f"pjs{t}")
            for i in range(4):
                nc.vector.tensor_add(pjs[:, i, :], pj[:, i, :], pj[:, 4 + i, :])
            for fc in range(3):
                for tcc in range(4):
                    pp = psbig.tile([128, 512], F32, tag="big", name=f"pp{t}")
                    for i in range(4):
                        nc.tensor.matmul(
                            pp[:],
                            pjs[:, i, tcc * 128 : (tcc + 1) * 128],
                            wp_t[t][:, 3 * i + fc, :],
                            start=(i == 0),
                            stop=(i == 3),
                        )
                    if t == 0:
                        acc[fc * 4 + tcc] = late["accp"].tile(
                            [128, 512], BF16, tag=f"acc{fc * 4 + tcc}",
                            name=f"acc{fc * 4 + tcc}",
                        )
                        nc.vector.tensor_copy(acc[fc * 4 + tcc][:], pp[:])
                    elif t == 1:
                        a = acc[fc * 4 + tcc]
                        nc.vector.tensor_add(a[:], a[:], pp[:])
                    else:
                        a = acc[fc * 4 + tcc]
                        nc.vector.tensor_add(a[:], a[:], pp[:])
                        ob = avsp.tile(
                            [128, 512], F32, tag="avs", name=f"ob{fc}_{tcc}"
                        )
                        nc.vector.tensor_tensor(
                            ob[:], a[:], bb_sb[:, fc * 512 : (fc + 1) * 512],
                            ALU.add,
                        )
                        nc.sync.dma_start(
                            out[
                                tcc * 128 : (tcc + 1) * 128,
                                fc * 512 : (fc + 1) * 512,
                            ],
                            ob[:],
                        )

        # ---------------- emission schedule ----------------
        with tc.tile_pool(name="wvp", bufs=1) as wvp:
            wvx_sb = wvp.tile([128, NCT, NHL * HD], BF16, tag="wvx")
            nc.sync.dma_start(wvx_sb[:], wvx.rearrange("(ct p) c -> p ct c", p=128))
            wvy_sb = wvp.tile([128, NCT, NHL * HD], BF16, tag="wvy")
            nc.sync.dma_start(wvy_sb[:], wvy.rearrange("(ct p) c -> p ct c", p=128))

            for p in head_pieces(0):
                p()
            for kt in range(4):
                emit_vtile(kt)

            fi1 = head_pieces(1)
            emit_attention(0, fi1, vzip=True)
            emit_a2a(0)
        late["wpp"] = ctx.enter_context(tc.tile_pool(name="wpp", bufs=1))
        late["pjp"] = ctx.enter_context(tc.tile_pool(name="pjp", bufs=1))
        late["pjsp"] = ctx.enter_context(tc.tile_pool(name="pjsp", bufs=1))
        late["accp"] = ctx.enter_context(tc.tile_pool(name="accp", bufs=1))
        emit_wp_load(0)

        fi2 = head_pieces(2)

        def extra1(qc):
            if qc == 0:
                emit_pj_load(0)
            elif qc == 1:
                emit_proj(0)
                emit_wp_load(1)

        emit_attention(1, fi2, extra_per_qc=extra1)
        emit_a2a(1)

        def extra2(qc):
            if qc == 0:
                emit_pj_load(1)
            elif qc == 1:
                emit_proj(1)
                emit_wp_load(2)

        emit_attention(2, [], extra_per_qc=extra2)
        emit_a2a(2)
        emit_pj_load(2)
        emit_proj(2)
    nc.compile()
    return nc


_NC_CACHE = {}


def _get_nc():
    if "nc" not in _NC_CACHE:
        _NC_CACHE["nc"] = build_nc()
    return _NC_CACHE["nc"]


def make_in_maps(x, y, pos, y_token_weights, Wqkv, Wkv, q_norm_w, k_norm_w, Wproj, bproj):
    f = np.float32
    bf = ml_dtypes.bfloat16
    pos = np.asarray(pos, dtype=f)
    c32 = pos[:, :, 0].T
    s32 = pos[:, :, 1].T
    csT = np.ascontiguousarray(np.concatenate([c32, c32], 0).astype(bf))   # [64, N]
    snT = np.ascontiguousarray(np.concatenate([-s32, s32], 0).astype(bf))  # [64, N]
    wq = np.asarray(q_norm_w, dtype=f)
    wk = np.asarray(k_norm_w, dtype=f)
    Wqkv = np.asarray(Wqkv, dtype=f)
    Wkv = np.asarray(Wkv, dtype=f)
    Wp = np.asarray(Wproj, dtype=f)

    wproj_rows = np.empty((NHL, 4, 128, C), dtype=f)
    for t in range(NHL):
        for i in range(4):
            h = 3 * i + t
            wproj_rows[t, i] = Wp[h * 128 : (h + 1) * 128, :]
    wproj_host = np.ascontiguousarray(wproj_rows.reshape(NHL * 4 * 128, C).astype(bf))

    in_maps = []
    for c in range(8):
        b, g = c // 4, c % 4
        heads = [3 * g + i for i in range(NHL)]
        qk_cols = []
        ky_cols = []
        vx_cols = []
        vy_cols = []
        for h in heads:
            qk_cols.append(Wqkv[:, h * HD : (h + 1) * HD] * wq[None, :])
            qk_cols.append(Wqkv[:, C + h * HD : C + (h + 1) * HD] * wk[None, :])
            ky_cols.append(Wkv[:, h * HD : (h + 1) * HD] * wk[None, :])
            vx_cols.append(Wqkv[:, 2 * C + h * HD : 2 * C + (h + 1) * HD])
            vy_cols.append(Wkv[:, C + h * HD : C + (h + 1) * HD])
        in_maps.append(
            {
                "xT": np.ascontiguousarray(np.asarray(x)[b].T.astype(bf)),
                "yT": np.ascontiguousarray(np.asarray(y)[b].T.astype(bf)),
                "wqk": np.ascontiguousarray(np.concatenate(qk_cols, axis=1).astype(bf)),
                "wky": np.ascontiguousarray(np.concatenate(ky_cols, axis=1).astype(bf)),
                "wvx": np.ascontiguousarray(np.concatenate(vx_cols, axis=1).astype(bf)),
                "wvy": np.ascontiguousarray(np.concatenate(vy_cols, axis=1).astype(bf)),
                "wproj": wproj_host,
                "cs": csT,
                "sn": snT,
                "ywT": np.ascontiguousarray(
                    np.asarray(y_token_weights)[b].reshape(M // 128, 128).T, dtype=f
                ),
                "bpr": np.asarray(bproj, dtype=f).reshape(1, C).astype(bf),
                "onesb": np.ones((128, 1), dtype=bf),
                "mska": np.full((128, 1), 1.0 if b == 0 else 0.0, dtype=f),
                "mskb": np.full((128, 1), 0.0 if b == 0 else 1.0, dtype=f),
            }
        )
    return in_maps


def kernel(x, y, pos, y_token_weights, Wqkv, Wkv, q_norm_w, k_norm_w, Wproj, bproj,
           _trace=False):
    x = np.asarray(x, dtype=np.float32)
    y = np.asarray(y, dtype=np.float32)
    pos = np.asarray(pos, dtype=np.float32)
    y_token_weights = np.asarray(y_token_weights, dtype=np.float32)
    nc = _get_nc()
    in_maps = make_in_maps(
        x, y, pos, y_token_weights,
        np.asarray(Wqkv), np.asarray(Wkv), np.asarray(q_norm_w),
        np.asarray(k_norm_w), np.asarray(Wproj), np.asarray(bproj),
    )
    res = run_bass_kernel_spmd(nc, in_maps, core_ids=list(range(8)), trace=_trace)
    outp = np.zeros((B, N, C), dtype=np.float32)
    for c in range(8):
        b, g = c // 4, c % 4
        outp[b, g * 512 : (g + 1) * 512, :] = res.results[c]["out"]
    if _trace:
        return outp, res
    return outp
